# revision 8
# baseline (speedup 1.0000x reference)
"""Trainium2 Bass kernel v3 for nn_MixtureOfExperts (top-2, E=8, D=1024, H=512).

Data-parallel over tokens (2048/core). Per core:
  Phase R: gates = x@Wg+bg on PE in exact fp32 (xt pre-transposed on host),
           top-2 via DVE max/max_index, w1 = sigmoid(g2-g1) on ACT written
           straight into the scatter pair tiles; dispatch positions via
           strict-upper rank matmul + running counts; (tokid, w) scattered
           inline to meta2.
  Phase E (expert quads, staged for long PE bursts + ACT table locality):
           gather rows of xb[e] = x + b2[e] (f16, host-precomputed; the b2
           shift through W1 is compensated in b1' = b1 - b2@W1, exact), PE
           f16 transposes, fp8 cast copies, fp8e4m3 DoubleRow W1, gelu,
           DoubleRow W2 (no bias matmuls - residual carries b2), y = psum +
           xg on DVE (accum mu), var = E[y^2]-mu^2 via TTR, rn' =
           (y-mu)*rstd*w scatter-ADDed into f16 out. Pad slots have w=0.
"""

import numpy as np
import ml_dtypes as md
import concourse.bass as bass
from concourse import mybir
from concourse.tile import TileContext
from concourse.masks import make_identity, make_upper_triangular
from concourse.vector_clock import ScopedClock

F32 = mybir.dt.float32
F16 = mybir.dt.float16
F8 = mybir.dt.float8e4
I32 = mybir.dt.int32
U32 = mybir.dt.uint32
AF = mybir.ActivationFunctionType
ALU = mybir.AluOpType
PM = mybir.MatmulPerfMode

T = 2048
D = 1024
H = 512
E = 8
G = T // 128
CAP = 640
ST = CAP // 128
NST = E * ST
XROW = 1024
LN_EPS = 1e-5
N_CORES = 8
QUAD = 4


def _patched_drain_and_barrier(self, tick_clock, wait_clock):
    nc = self.nc
    probe = nc.sync.nop(nofuse=True, hint="pre_drain_wait")
    wait_clock.add_sem_waits(probe.ins, ScopedClock({None: tick_clock.global_clock}))
    si = probe.ins.sync_info
    if si is not None and si.on_wait and len(si.on_wait) > 1:
        waits = list(si.on_wait)
        probe.ins.sync_info = mybir.SyncInfo(
            on_wait=[waits[0]], on_update=list(si.on_update))
        for w in waits[1:]:
            n2 = nc.sync.nop(nofuse=True, hint="pre_drain_wait")
            n2.ins.sync_info = mybir.SyncInfo(on_wait=[w], on_update=[])
    nc.sync.drain()
    nc.all_engine_barrier()
    assert self.sems is not None
    popped = nc._tile_sem_poison_stack.pop()
    assert popped is self._sem_poison
    nc.clear_and_free_semaphores(list(self.sems.allocated().values()))
    nc.all_engine_barrier()


def _apply_tile_patch():
    TileContext._drain_and_barrier = _patched_drain_and_barrier


def _legalize_multiwait(nc):
    for f in nc.m.functions:
        for bb in f.blocks:
            insts = list(bb.instructions)
            out, changed = [], False
            for inst in insts:
                si = inst.sync_info
                cap = 2 if isinstance(inst, mybir.InstEventSemaphore) else 1
                if si is not None and si.on_wait and len(si.on_wait) > cap:
                    waits = list(si.on_wait)
                    for w in waits[cap:]:
                        nop = mybir.InstNoOp(
                            name=nc.get_next_instruction_name(), ins=[], outs=[])
                        nop.engine = inst.engine
                        nop.bass_nofuse = True
                        nop.sync_info = mybir.SyncInfo(on_wait=[w], on_update=[])
                        nc.register_instruction(nop)
                        out.append(nop)
                    inst.sync_info = mybir.SyncInfo(
                        on_wait=waits[:cap], on_update=list(si.on_update))
                    changed = True
                out.append(inst)
            if changed:
                bb.instructions = out


def build_kernel():
    nc = bass.Bass()

    xt = nc.dram_tensor("xt", [128, 8, T], F32, kind="ExternalInput")
    xb = nc.dram_tensor("xb", [E * T, XROW], F16, kind="ExternalInput")
    wgt = nc.dram_tensor("wgt", [128, 8, E], F32, kind="ExternalInput")
    bg = nc.dram_tensor("bg", [1, E], F32, kind="ExternalInput")
    w1 = nc.dram_tensor("w1", [E, 128, 8, H], F8, kind="ExternalInput")
    b1t = nc.dram_tensor("b1t", [128, E, 4], F32, kind="ExternalInput")
    w2 = nc.dram_tensor("w2", [E, 128, 4, D], F8, kind="ExternalInput")
    w2s = nc.dram_tensor("w2s", [E, 128, 4, 1], F8, kind="ExternalInput")
    outb = nc.dram_tensor("outb", [T, D], F16, kind="ExternalOutput")
    zrow = nc.dram_tensor("zrow", [1, D], F16, kind="ExternalInput")

    with TileContext(nc) as tc:
        with (
            tc.tile_pool(name="const", bufs=1) as cpool,
            tc.tile_pool(name="resident", bufs=1) as rpool,
            tc.tile_pool(name="work", bufs=3) as wpool,
            tc.tile_pool(name="xgp", bufs=2 + ST * QUAD) as xgpool,
            tc.tile_pool(name="xtp", bufs=1 + QUAD) as xtpool,
            tc.tile_pool(name="hp", bufs=1 + QUAD) as hpool,
            tc.tile_pool(name="wts", bufs=2) as wtpool,
            tc.tile_pool(name="psT", bufs=1, space="PSUM") as psT,
            tc.tile_pool(name="psH", bufs=1, space="PSUM") as psH,
            tc.tile_pool(name="psY", bufs=2, space="PSUM") as psY,
            tc.tile_pool(name="dram", bufs=1, space="DRAM") as dpool,
        ):
            # ---------------- constants ----------------
            ident16 = cpool.tile([128, 128], F16)
            make_identity(nc, ident16[:])
            ustrict = cpool.tile([128, 128], F32)
            make_upper_triangular(nc, ustrict[:], val=1.0, diag=False)
            ones_col = cpool.tile([128, 1], F32)
            nc.vector.memset(ones_col[:], 1.0)
            ones_row1 = cpool.tile([1, 128], F32)
            nc.vector.memset(ones_row1[:], 1.0)
            tokid = cpool.tile([128, G], I32)
            nc.gpsimd.iota(tokid[:], pattern=[[128, G]], base=0, channel_multiplier=1)
            basecap_i = cpool.tile([1, E], I32)
            nc.gpsimd.iota(basecap_i[:], pattern=[[CAP, E]], base=0,
                           channel_multiplier=0)
            basecap8 = cpool.tile([1, E], F32)
            nc.vector.tensor_copy(basecap8[:], basecap_i[:])
            eidx_i = cpool.tile([128, E], I32)
            nc.gpsimd.iota(eidx_i[:], pattern=[[1, E]], base=0,
                           channel_multiplier=0)
            eidx = cpool.tile([128, E], F32)
            nc.vector.tensor_copy(eidx[:], eidx_i[:])
            eps_col = cpool.tile([128, 1], F32)
            nc.vector.memset(eps_col[:], LN_EPS)
            ones8c = cpool.tile([128, 2, 1], F8)
            nc.vector.memset(ones8c[:], 1.0)
            magic_col = cpool.tile([128, 1], I32)
            nc.vector.memset(magic_col[:], 0x5ef759df)

            wg_sb = rpool.tile([128, 8, E], F32)
            nc.sync.dma_start(out=wg_sb[:], in_=wgt[:, :, :])
            bg_sb = rpool.tile([1, E], F32)
            nc.sync.dma_start(out=bg_sb[:], in_=bg[:, :])
            b1_sb = rpool.tile([128, E, 4], F32)
            nc.sync.dma_start(out=b1_sb[:], in_=b1t[:, :, :])


            meta2 = dpool.tile([NST * 128, 2], I32)
            zmeta = wpool.tile([NST, 256], I32, tag="zmeta")
            nc.vector.memset(zmeta[:], 0)
            nc.sync.dma_start(
                out=meta2[:].rearrange("(s q) two -> s (q two)", q=128),
                in_=zmeta[:])

            # zero the f16 output buffer in one broadcast DMA (overlaps router)
            nc.gpsimd.dma_start(out=outb[:, :], in_=zrow[:, :].to_broadcast([T, D]))

            # preload first quad's weights
            w1_sbs, w2_sbs = {}, {}
            for e in range(QUAD):
                w1_sbs[e] = wtpool.tile([128, 8, H], F8, tag=f"w1_sb{e % QUAD}", name=f"w1s{e}")
                nc.scalar.dma_start(out=w1_sbs[e][:], in_=w1[e])
                w2_sbs[e] = wtpool.tile([128, 4, D], F8, tag=f"w2_sb{e % QUAD}", name=f"w2s{e}")
                nc.sync.dma_start(out=w2_sbs[e][:], in_=w2[e])

            # ---------------- Phase R: router (runp-in-PSUM prefix) ----
            onesmat = cpool.tile([128, 128], F32)
            nc.vector.memset(onesmat[:], 1.0)
            mgs = []
            pos_i = [rpool.tile([128, G], I32, name=f"pos{k}_i") for k in range(2)]
            pairs = [rpool.tile([128, G, 2], I32, name=f"pairs{k}") for k in range(2)]
            for k in range(2):
                nc.vector.tensor_copy(pairs[k][:, :, 0], tokid[:])

            for g in range(G):
                xtg = wpool.tile([128, 8, 128], F32, tag="xtg", bufs=5)
                eng = nc.sync if g % 8 < 5 else nc.scalar
                eng.dma_start(out=xtg[:], in_=xt[:, :, g * 128:(g + 1) * 128])
                if g % 2 == 0:
                    gps = psY.tile([128, D], F32, tag="yps", name="gps")[:, :E]
                else:
                    gps = psT.tile([128, 1024], F16, tag="tp", bufs=1,
                                   name="gpsb").bitcast(F32)[:, :E]
                for dc in range(8):
                    nc.tensor.matmul(gps[:], lhsT=xtg[:, dc, :], rhs=wg_sb[:, dc, :],
                                     start=(dc == 0), stop=False)
                nc.tensor.matmul(gps[:], lhsT=ones_row1[:], rhs=bg_sb[:, :],
                                 start=False, stop=True)
                mx8 = wpool.tile([128, 8], F32, tag="mx8", bufs=5)
                nc.vector.max(out=mx8[:], in_=gps[:])
                ix8 = wpool.tile([128, 8], U32, tag="ix8", bufs=5)
                nc.vector.max_index(out=ix8[:], in_max=mx8[:], in_values=gps[:])
                dgap = wpool.tile([128, 1], F32, tag="dgap", bufs=5)
                nc.vector.tensor_sub(dgap[:], mx8[:, 1:2], mx8[:, 0:1])
                w1c = pairs[1][:, g, 1:2].bitcast(F32)
                nc.scalar.activation(w1c, dgap[:], AF.Sigmoid)
                nc.scalar.activation(pairs[0][:, g, 1:2].bitcast(F32), w1c,
                                     AF.Copy, bias=1.0, scale=-1.0)
                e0c = wpool.tile([128, 1], F32, tag="e0c", bufs=5)
                nc.vector.tensor_copy(e0c[:], ix8[:, 0:1])
                e1c = wpool.tile([128, 1], F32, tag="e1c", bufs=5)
                nc.vector.tensor_copy(e1c[:], ix8[:, 1:2])
                m0g = wpool.tile([128, E], F32, tag="m0g", bufs=5)
                nc.vector.tensor_tensor(out=m0g[:],
                                        in0=e0c[:].to_broadcast([128, E]),
                                        in1=eidx[:], op=ALU.is_equal)
                m1g = wpool.tile([128, E], F32, tag="m1g", bufs=5)
                nc.vector.tensor_tensor(out=m1g[:],
                                        in0=e1c[:].to_broadcast([128, E]),
                                        in1=eidx[:], op=ALU.is_equal)
                mg = wpool.tile([128, E], F32, tag="mg", bufs=G + 1,
                                name=f"mg{g}")
                nc.vector.tensor_add(mg[:], m0g[:], m1g[:])
                mgs.append(mg)
                pwg = psH.tile([128, 512], F32, tag="hpsa", name="pwg",
                               bufs=2)[:, :E]
                nc.tensor.matmul(pwg[:], lhsT=ustrict[:], rhs=mg[:],
                                 start=True, stop=False)
                for gp in range(g):
                    nc.tensor.matmul(pwg[:], lhsT=onesmat[:], rhs=mgs[gp][:],
                                     start=False, stop=False)
                nc.tensor.matmul(pwg[:], lhsT=ones_row1[:], rhs=basecap8[:, :],
                                 start=False, stop=True)
                for k, mk in ((0, m0g), (1, m1g)):
                    pk = wpool.tile([128, E], F32, tag="pk", bufs=5)
                    nc.vector.tensor_mul(pk[:], pwg[:], mk[:])
                    with nc.allow_low_precision(reason="integer-valued positions"):
                        nc.vector.tensor_reduce(pos_i[k][:, g:g + 1], pk[:],
                                                axis=mybir.AxisListType.X,
                                                op=ALU.add)
                    nc.gpsimd.indirect_dma_start(
                        out=meta2[:, :],
                        out_offset=bass.IndirectOffsetOnAxis(
                            ap=pos_i[k][:, g:g + 1], axis=0),
                        in_=pairs[k][:, g, :],
                        in_offset=None,
                    )

            meta_sb = rpool.tile([128, NST], I32)
            nc.sync.dma_start(
                out=meta_sb[:],
                in_=meta2[:, 0:1].rearrange("(s q) one -> q (s one)", q=128))
            wcol = rpool.tile([128, E, ST], F32)
            nc.sync.dma_start(
                out=wcol[:].bitcast(I32),
                in_=meta2[:, 1:2].rearrange("(e s q) one -> q e (s one)",
                                            q=128, s=ST))

            # ---------------- Phase E: software-pipelined expert stages ----
            xTs, hss, xgss, eixs = {}, {}, {}, {}

            def emit_A0(e):
                xT = xtpool.tile([128, 8, CAP], F8, tag="xT", name=f"xT{e}")
                eix = wpool.tile([128, ST], I32, tag="eix", bufs=1 + QUAD,
                                 name=f"eix{e}")
                nc.vector.tensor_scalar_add(
                    eix[:], meta_sb[:, e * ST:(e + 1) * ST], float(e * T))
                eixs[e] = eix
                xgs = []
                for s in range(ST):
                    xg = xgpool.tile([128, 1024], F16, tag="xg", name=f"xg{e}_{s}")
                    nc.gpsimd.indirect_dma_start(
                        out=xg[:], out_offset=None, in_=xb[:, :],
                        in_offset=bass.IndirectOffsetOnAxis(
                            ap=eix[:, s:s + 1], axis=0),
                    )
                    xgs.append(xg)
                    tp = psT.tile([128, 1024], F16, tag="tp", bufs=1, name="tp")
                    for dc in range(8):
                        nc.tensor.transpose(tp[:, dc * 128:(dc + 1) * 128],
                                            xg[:, dc * 128:(dc + 1) * 128],
                                            ident16[:])
                    cp_out = xT[:, :, s * 128:(s + 1) * 128]
                    cp_in = tp[:].rearrange("p (dc t) -> p dc t", dc=8)
                    if s % 2 == 1:
                        nc.vector.tensor_copy(cp_out, cp_in)
                    else:
                        nc.scalar.copy(cp_out, cp_in)
                xTs[e] = xT
                xgss[e] = xgs

            def emit_A1(e):
                xT, w1_sb = xTs[e], w1_sbs[e]
                h_sb = hpool.tile([128, 4, CAP], F8, tag="h_sb", name=f"h{e}")
                hss[e] = h_sb
                for hc in range(4):
                    hpsa = psH.tile([128, 512], F32, tag="hpsa", bufs=2,
                                    name=f"hpsa{e}_{hc}")
                    for c in range(4):
                        nc.tensor.matmul(
                            hpsa[:],
                            lhsT=w1_sb[:, 2 * c:2 * c + 2,
                                       hc * 128:(hc + 1) * 128],
                            rhs=xT[:, 2 * c:2 * c + 2, 0:512],
                            start=(c == 0), stop=(c == 3),
                            perf_mode=PM.DoubleRow)
                    hpsb = psH.tile([128, 128], F32, tag="hpsb", bufs=1,
                                    name=f"hpsb{e}_{hc}")
                    for c in range(4):
                        nc.tensor.matmul(
                            hpsb[:],
                            lhsT=w1_sb[:, 2 * c:2 * c + 2,
                                       hc * 128:(hc + 1) * 128],
                            rhs=xT[:, 2 * c:2 * c + 2, 512:CAP],
                            start=(c == 0), stop=(c == 3),
                            perf_mode=PM.DoubleRow)
                    nc.scalar.activation(h_sb[:, hc, 0:512], hpsa[:],
                                         AF.Gelu, bias=b1_sb[:, e, hc:hc + 1],
                                         scale=1.0)
                    nc.scalar.activation(h_sb[:, hc, 512:CAP], hpsb[:],
                                         AF.Gelu, bias=b1_sb[:, e, hc:hc + 1],
                                         scale=1.0)

            def emit_B(e):
                w2_sb = w2_sbs[e]
                h_sb = hss[e]
                variant2 = (e % 2 == 0)
                if variant2:
                    w2s_sb = wtpool.tile([128, 4, 1], F8, tag="w2s_sb",
                                         bufs=2, name=f"w2s_sb{e}")
                    nc.scalar.dma_start(out=w2s_sb[:], in_=w2s[e])
                sy5 = wpool.tile([128, ST], F32, tag="sy5", bufs=2, name="sy5")
                ss5 = wpool.tile([128, ST], F32, tag="ss5", bufs=2, name="ss5")
                negmu5 = wpool.tile([128, ST], F32, tag="negmu5", bufs=2,
                                    name="negmu5")
                y_sbs = []
                for s in range(ST):
                    yps = psY.tile([128, D], F32, tag="yps", name="yps")
                    for nch in range(2):
                        for c in range(2):
                            nc.tensor.matmul(
                                yps[:, nch * 512:(nch + 1) * 512],
                                lhsT=h_sb[:, 2 * c:2 * c + 2,
                                          s * 128:(s + 1) * 128],
                                rhs=w2_sb[:, 2 * c:2 * c + 2,
                                          nch * 512:(nch + 1) * 512],
                                start=(c == 0), stop=(c == 1),
                                perf_mode=PM.DoubleRow)
                    y_sb = wpool.tile([128, D], F16, tag="y_sb", bufs=ST + 1,
                                      name="y_sb")
                    if variant2:
                        ymu = psH.tile([128, 128], F32, tag="hpsb", bufs=1,
                                       name="ymu")[:, 0:1]
                        for c in range(2):
                            nc.tensor.matmul(
                                ymu[:],
                                lhsT=h_sb[:, 2 * c:2 * c + 2,
                                          s * 128:(s + 1) * 128],
                                rhs=w2s_sb[:, 2 * c:2 * c + 2, :],
                                start=(c == 0), stop=False,
                                perf_mode=PM.DoubleRow)
                        for c in range(4):
                            nc.tensor.matmul(
                                ymu[:],
                                lhsT=xTs[e][:, 2 * c:2 * c + 2,
                                            s * 128:(s + 1) * 128],
                                rhs=ones8c[:],
                                start=False, stop=(c == 3),
                                perf_mode=PM.DoubleRow)
                        nc.scalar.copy(y_sb[:], yps[:])
                        nc.gpsimd.indirect_dma_start(
                            out=y_sb[:], out_offset=None, in_=xb[:, :],
                            in_offset=bass.IndirectOffsetOnAxis(
                                ap=eixs[e][:, s:s + 1], axis=0),
                            compute_op=ALU.add,
                        )
                        nc.vector.tensor_scalar_mul(
                            negmu5[:, s:s + 1], ymu[:], -1.0 / D)
                    else:
                        nc.vector.scalar_tensor_tensor(
                            out=y_sb[:], in0=yps[:], scalar=0.0,
                            in1=xgss[e][s][:, 0:1024],
                            op0=ALU.add, op1=ALU.add,
                            accum_out=sy5[:, s:s + 1])
                    sqscr = wpool.tile([128, D], F16, tag="sqscr", bufs=3,
                                       name="sqscr")
                    if (e + s) % 2 == 0:
                        nc.scalar.activation(sqscr[:], y_sb[:], AF.Square,
                                             accum_out=ss5[:, s:s + 1])

                    else:
                        nc.vector.scalar_tensor_tensor(
                            out=sqscr[:], in0=y_sb[:], scalar=0.0,
                            in1=y_sb[:], op0=ALU.add, op1=ALU.mult,
                            accum_out=ss5[:, s:s + 1])
                    y_sbs.append(y_sb)
                if not variant2:
                    nc.vector.tensor_scalar_mul(negmu5[:], sy5[:], -1.0 / D)
                msq = wpool.tile([128, ST], F32, tag="msq", name="msq")
                nc.vector.tensor_mul(msq[:], negmu5[:], negmu5[:])
                varp = wpool.tile([128, ST], F32, tag="varp", name="varp")
                nc.vector.scalar_tensor_tensor(
                    out=varp[:], in0=ss5[:], scalar=1.0 / D, in1=msq[:],
                    op0=ALU.mult, op1=ALU.subtract)
                vh5 = wpool.tile([128, ST], F32, tag="vh5", name="vh5")
                nc.vector.tensor_scalar(vh5[:], varp[:], 0.5, 0.5 * LN_EPS,
                                        op0=ALU.mult, op1=ALU.add)
                ib = wpool.tile([128, ST], I32, tag="ib", name="ib")
                nc.vector.tensor_scalar(ib[:], vh5[:].bitcast(I32), 1, 0,
                                        op0=ALU.logical_shift_right,
                                        op1=ALU.logical_shift_right)
                yb = wpool.tile([128, ST], I32, tag="yb", name="yb")
                nc.vector.tensor_tensor(out=yb[:],
                                        in0=magic_col[:].to_broadcast([128, ST]),
                                        in1=ib[:], op=ALU.subtract)
                rstd5 = yb[:].bitcast(F32)
                for _ in range(1):
                    ya = wpool.tile([128, ST], F32, tag="ya", name="ya")
                    nc.vector.tensor_mul(ya[:], rstd5, rstd5)
                    nc.vector.tensor_mul(ya[:], ya[:], vh5[:])
                    nc.vector.tensor_scalar(ya[:], ya[:], -1.0, 1.5,
                                            op0=ALU.mult, op1=ALU.add)
                    nc.vector.tensor_mul(rstd5, rstd5, ya[:])
                rsw5 = wpool.tile([128, ST], F32, tag="rsw5", bufs=2, name="rsw5")
                nc.vector.tensor_mul(rsw5[:], rstd5, wcol[:, e, :])
                for s in range(ST):
                    S = e * ST + s
                    rn = wpool.tile([128, D], F16, tag="rn", bufs=4, name="rn")
                    nc.vector.tensor_scalar(
                        rn[:], y_sbs[s][:], negmu5[:, s:s + 1],
                        rsw5[:, s:s + 1], op0=ALU.add, op1=ALU.mult)
                    nc.gpsimd.indirect_dma_start(
                        out=outb[:, :],
                        out_offset=bass.IndirectOffsetOnAxis(
                            ap=meta_sb[:, S:S + 1], axis=0),
                        in_=rn[:], in_offset=None,
                        compute_op=ALU.add,
                    )

            def prefetch_weights(ne):
                w1_sbs[ne] = wtpool.tile([128, 8, H], F8,
                                         tag=f"w1_sb{ne % QUAD}",
                                         name=f"w1s{ne}")
                nc.sync.dma_start(out=w1_sbs[ne][:], in_=w1[ne])
                w2_sbs[ne] = wtpool.tile([128, 4, D], F8,
                                         tag=f"w2_sb{ne % QUAD}",
                                         name=f"w2s{ne}")
                nc.sync.dma_start(out=w2_sbs[ne][:], in_=w2[ne])

            emit_A0(0)
            emit_A0(1)
            emit_A1(0)
            for e in range(E):
                if e + 2 < E:
                    if e + 2 >= QUAD:
                        prefetch_weights(e + 2)
                    emit_A0(e + 2)
                if e + 1 < E:
                    emit_A1(e + 1)
                emit_B(e)

    _legalize_multiwait(nc)
    return nc


def make_in_maps(inputs):
    x = np.ascontiguousarray(
        np.asarray(inputs["x"], dtype=np.float32).reshape(-1, D))
    Wg = np.asarray(inputs["Wg"], dtype=np.float32)
    bgv = np.asarray(inputs["bg"], dtype=np.float32)
    W1 = np.asarray(inputs["W1"], dtype=np.float32)
    b1 = np.asarray(inputs["b1"], dtype=np.float32)
    W2 = np.asarray(inputs["W2"], dtype=np.float32)
    b2v = np.asarray(inputs["b2"], dtype=np.float32)

    wgt = np.ascontiguousarray(Wg.reshape(8, 128, E).transpose(1, 0, 2))
    # b1' = b1 - b2 @ W1  (compensates the b2 shift folded into the gather)
    b1p = b1 - np.einsum('ed,edh->eh', b2v, W1)
    b1t = np.ascontiguousarray(b1p.reshape(E, 4, 128).transpose(2, 0, 1))
    w1f8 = np.ascontiguousarray(
        W1.reshape(E, 8, 128, H).transpose(0, 2, 1, 3)).astype(md.float8_e4m3)
    w2f8 = np.ascontiguousarray(
        W2.reshape(E, 4, 128, D).transpose(0, 2, 1, 3)).astype(md.float8_e4m3)
    w2sum = np.ascontiguousarray(
        W2.sum(axis=2).reshape(E, 4, 128, 1).transpose(0, 2, 1, 3)
    ).astype(md.float8_e4m3)

    shared = {
        "wgt": wgt,
        "bg": bgv.reshape(1, E),
        "w1": w1f8,
        "b1t": b1t,
        "w2": w2f8,
        "w2s": w2sum,
        "zrow": np.zeros((1, D), np.float16),
    }
    maps = []
    for c in range(N_CORES):
        xs = x[c * T:(c + 1) * T]
        xtc = np.ascontiguousarray(xs.reshape(T, 8, 128).transpose(2, 1, 0))
        xbe = np.zeros((E * T, XROW), np.float16)
        for e in range(E):
            xbe[e * T:(e + 1) * T] = (xs + b2v[e]).astype(np.float16)
        maps.append(dict(shared, xt=xtc, xb=xbe))
    return maps


_CACHED = {}


def kernel(**inputs):
    _apply_tile_patch()
    from concourse.bass_utils import run_bass_kernel_spmd

    if "nc" not in _CACHED:
        _CACHED["nc"] = build_kernel()
    nc = _CACHED["nc"]
    in_maps = make_in_maps(inputs)
    res = run_bass_kernel_spmd(nc, in_maps, core_ids=list(range(N_CORES)),
                               trace=False)
    out = np.concatenate(
        [np.asarray(res.results[c]["outb"]).astype(np.float32)
         for c in range(N_CORES)], axis=0)
    xshape = np.asarray(inputs["x"]).shape
    return out.reshape(xshape)


# revision 9
# speedup vs baseline: 1.0122x; 1.0122x over previous
"""Trainium2 Bass kernel v3 for nn_MixtureOfExperts (top-2, E=8, D=1024, H=512).

Data-parallel over tokens (2048/core). Per core:
  Phase R: gates = x@Wg+bg on PE in exact fp32 (xt pre-transposed on host),
           top-2 via DVE max/max_index, w1 = sigmoid(g2-g1) on ACT written
           straight into the scatter pair tiles; dispatch positions via
           strict-upper rank matmul + running counts; (tokid, w) scattered
           inline to meta2.
  Phase E (expert quads, staged for long PE bursts + ACT table locality):
           gather rows of xb[e] = x + b2[e] (f16, host-precomputed; the b2
           shift through W1 is compensated in b1' = b1 - b2@W1, exact), PE
           f16 transposes, fp8 cast copies, fp8e4m3 DoubleRow W1, gelu,
           DoubleRow W2 (no bias matmuls - residual carries b2), y = psum +
           xg on DVE (accum mu), var = E[y^2]-mu^2 via TTR, rn' =
           (y-mu)*rstd*w scatter-ADDed into f16 out. Pad slots have w=0.
"""

import numpy as np
import ml_dtypes as md
import concourse.bass as bass
from concourse import mybir
from concourse.tile import TileContext
from concourse.masks import make_identity, make_upper_triangular
from concourse.vector_clock import ScopedClock

F32 = mybir.dt.float32
F16 = mybir.dt.float16
F8 = mybir.dt.float8e4
I32 = mybir.dt.int32
U32 = mybir.dt.uint32
AF = mybir.ActivationFunctionType
ALU = mybir.AluOpType
PM = mybir.MatmulPerfMode

T = 2048
D = 1024
H = 512
E = 8
G = T // 128
CAP = 640
ST = CAP // 128
NST = E * ST
XROW = 1024
LN_EPS = 1e-5
N_CORES = 8
QUAD = 4


def _patched_drain_and_barrier(self, tick_clock, wait_clock):
    nc = self.nc
    probe = nc.sync.nop(nofuse=True, hint="pre_drain_wait")
    wait_clock.add_sem_waits(probe.ins, ScopedClock({None: tick_clock.global_clock}))
    si = probe.ins.sync_info
    if si is not None and si.on_wait and len(si.on_wait) > 1:
        waits = list(si.on_wait)
        probe.ins.sync_info = mybir.SyncInfo(
            on_wait=[waits[0]], on_update=list(si.on_update))
        for w in waits[1:]:
            n2 = nc.sync.nop(nofuse=True, hint="pre_drain_wait")
            n2.ins.sync_info = mybir.SyncInfo(on_wait=[w], on_update=[])
    nc.sync.drain()
    nc.all_engine_barrier()
    assert self.sems is not None
    popped = nc._tile_sem_poison_stack.pop()
    assert popped is self._sem_poison
    nc.clear_and_free_semaphores(list(self.sems.allocated().values()))
    nc.all_engine_barrier()


def _apply_tile_patch():
    TileContext._drain_and_barrier = _patched_drain_and_barrier


def _legalize_multiwait(nc):
    for f in nc.m.functions:
        for bb in f.blocks:
            insts = list(bb.instructions)
            out, changed = [], False
            for inst in insts:
                si = inst.sync_info
                cap = 2 if isinstance(inst, mybir.InstEventSemaphore) else 1
                if si is not None and si.on_wait and len(si.on_wait) > cap:
                    waits = list(si.on_wait)
                    for w in waits[cap:]:
                        nop = mybir.InstNoOp(
                            name=nc.get_next_instruction_name(), ins=[], outs=[])
                        nop.engine = inst.engine
                        nop.bass_nofuse = True
                        nop.sync_info = mybir.SyncInfo(on_wait=[w], on_update=[])
                        nc.register_instruction(nop)
                        out.append(nop)
                    inst.sync_info = mybir.SyncInfo(
                        on_wait=waits[:cap], on_update=list(si.on_update))
                    changed = True
                out.append(inst)
            if changed:
                bb.instructions = out


def build_kernel():
    nc = bass.Bass()

    xt = nc.dram_tensor("xt", [128, 8, T], F32, kind="ExternalInput")
    xb = nc.dram_tensor("xb", [E * T, XROW], F16, kind="ExternalInput")
    wgt = nc.dram_tensor("wgt", [128, 8, E], F32, kind="ExternalInput")
    bg = nc.dram_tensor("bg", [1, E], F32, kind="ExternalInput")
    w1 = nc.dram_tensor("w1", [E, 128, 8, H], F8, kind="ExternalInput")
    b1t = nc.dram_tensor("b1t", [128, E, 4], F32, kind="ExternalInput")
    w2 = nc.dram_tensor("w2", [E, 128, 4, D], F8, kind="ExternalInput")
    w2s = nc.dram_tensor("w2s", [E, 128, 4, 1], F8, kind="ExternalInput")
    outb = nc.dram_tensor("outb", [T, D], F16, kind="ExternalOutput")
    zrow = nc.dram_tensor("zrow", [1, D], F16, kind="ExternalInput")

    with TileContext(nc) as tc:
        with (
            tc.tile_pool(name="const", bufs=1) as cpool,
            tc.tile_pool(name="resident", bufs=1) as rpool,
            tc.tile_pool(name="work", bufs=3) as wpool,
            tc.tile_pool(name="xgp", bufs=2 + ST * QUAD) as xgpool,
            tc.tile_pool(name="xtp", bufs=1 + QUAD) as xtpool,
            tc.tile_pool(name="hp", bufs=1 + QUAD) as hpool,
            tc.tile_pool(name="wts", bufs=2) as wtpool,
            tc.tile_pool(name="psT", bufs=1, space="PSUM") as psT,
            tc.tile_pool(name="psH", bufs=1, space="PSUM") as psH,
            tc.tile_pool(name="psY", bufs=2, space="PSUM") as psY,
            tc.tile_pool(name="dram", bufs=1, space="DRAM") as dpool,
        ):
            # ---------------- constants ----------------
            ident16 = cpool.tile([128, 128], F16)
            make_identity(nc, ident16[:])
            ustrict = cpool.tile([128, 128], F32)
            make_upper_triangular(nc, ustrict[:], val=1.0, diag=False)
            ones_col = cpool.tile([128, 1], F32)
            nc.vector.memset(ones_col[:], 1.0)
            ones_row1 = cpool.tile([1, 128], F32)
            nc.vector.memset(ones_row1[:], 1.0)
            tokid = cpool.tile([128, G], I32)
            nc.gpsimd.iota(tokid[:], pattern=[[128, G]], base=0, channel_multiplier=1)
            basecap_i = cpool.tile([1, E], I32)
            nc.gpsimd.iota(basecap_i[:], pattern=[[CAP, E]], base=0,
                           channel_multiplier=0)
            basecap8 = cpool.tile([1, E], F32)
            nc.vector.tensor_copy(basecap8[:], basecap_i[:])
            eidx_i = cpool.tile([128, E], I32)
            nc.gpsimd.iota(eidx_i[:], pattern=[[1, E]], base=0,
                           channel_multiplier=0)
            eidx = cpool.tile([128, E], F32)
            nc.vector.tensor_copy(eidx[:], eidx_i[:])
            eps_col = cpool.tile([128, 1], F32)
            nc.vector.memset(eps_col[:], LN_EPS)
            ones8c = cpool.tile([128, 2, 1], F8)
            nc.vector.memset(ones8c[:], 1.0)
            magic_col = cpool.tile([128, 1], I32)
            nc.vector.memset(magic_col[:], 0x5ef759df)

            wg_sb = rpool.tile([128, 8, E], F32)
            nc.sync.dma_start(out=wg_sb[:], in_=wgt[:, :, :])
            bg_sb = rpool.tile([1, E], F32)
            nc.sync.dma_start(out=bg_sb[:], in_=bg[:, :])
            b1_sb = rpool.tile([128, E, 4], F32)
            nc.sync.dma_start(out=b1_sb[:], in_=b1t[:, :, :])


            meta2 = dpool.tile([NST * 128, 2], I32)
            zmeta = wpool.tile([NST, 256], I32, tag="zmeta")
            nc.vector.memset(zmeta[:], 0)
            nc.sync.dma_start(
                out=meta2[:].rearrange("(s q) two -> s (q two)", q=128),
                in_=zmeta[:])

            # zero the f16 output buffer in one broadcast DMA (overlaps router)
            nc.gpsimd.dma_start(out=outb[:, :], in_=zrow[:, :].to_broadcast([T, D]))

            # preload first quad's weights
            w1_sbs, w2_sbs = {}, {}
            for e in range(QUAD):
                w1_sbs[e] = wtpool.tile([128, 8, H], F8, tag=f"w1_sb{e % QUAD}", name=f"w1s{e}")
                nc.scalar.dma_start(out=w1_sbs[e][:], in_=w1[e])
                w2_sbs[e] = wtpool.tile([128, 4, D], F8, tag=f"w2_sb{e % QUAD}", name=f"w2s{e}")
                nc.sync.dma_start(out=w2_sbs[e][:], in_=w2[e])

            # ---------------- Phase R: router (runp-in-PSUM prefix) ----
            onesmat = cpool.tile([128, 128], F32)
            nc.vector.memset(onesmat[:], 1.0)
            mgs = []
            pos_i = [rpool.tile([128, G], I32, name=f"pos{k}_i") for k in range(2)]
            pairs = [rpool.tile([128, G, 2], I32, name=f"pairs{k}") for k in range(2)]
            for k in range(2):
                nc.vector.tensor_copy(pairs[k][:, :, 0], tokid[:])

            for g in range(G):
                xtg = wpool.tile([128, 8, 128], F32, tag="xtg", bufs=5)
                eng = nc.sync if g % 8 < 5 else nc.scalar
                eng.dma_start(out=xtg[:], in_=xt[:, :, g * 128:(g + 1) * 128])
                if g % 2 == 0:
                    gps = psY.tile([128, D], F32, tag="yps", name="gps")[:, :E]
                else:
                    gps = psT.tile([128, 1024], F16, tag="tp", bufs=1,
                                   name="gpsb").bitcast(F32)[:, :E]
                for dc in range(8):
                    nc.tensor.matmul(gps[:], lhsT=xtg[:, dc, :], rhs=wg_sb[:, dc, :],
                                     start=(dc == 0), stop=False)
                nc.tensor.matmul(gps[:], lhsT=ones_row1[:], rhs=bg_sb[:, :],
                                 start=False, stop=True)
                mx8 = wpool.tile([128, 8], F32, tag="mx8", bufs=5)
                nc.vector.max(out=mx8[:], in_=gps[:])
                ix8 = wpool.tile([128, 8], U32, tag="ix8", bufs=5)
                nc.vector.max_index(out=ix8[:], in_max=mx8[:], in_values=gps[:])
                dgap = wpool.tile([128, 1], F32, tag="dgap", bufs=5)
                nc.vector.tensor_sub(dgap[:], mx8[:, 1:2], mx8[:, 0:1])
                w1c = pairs[1][:, g, 1:2].bitcast(F32)
                nc.scalar.activation(w1c, dgap[:], AF.Sigmoid)
                nc.scalar.activation(pairs[0][:, g, 1:2].bitcast(F32), w1c,
                                     AF.Copy, bias=1.0, scale=-1.0)
                e0c = wpool.tile([128, 1], F32, tag="e0c", bufs=5)
                nc.vector.tensor_copy(e0c[:], ix8[:, 0:1])
                e1c = wpool.tile([128, 1], F32, tag="e1c", bufs=5)
                nc.vector.tensor_copy(e1c[:], ix8[:, 1:2])
                m0g = wpool.tile([128, E], F32, tag="m0g", bufs=5)
                nc.vector.tensor_tensor(out=m0g[:],
                                        in0=e0c[:].to_broadcast([128, E]),
                                        in1=eidx[:], op=ALU.is_equal)
                m1g = wpool.tile([128, E], F32, tag="m1g", bufs=5)
                nc.vector.tensor_tensor(out=m1g[:],
                                        in0=e1c[:].to_broadcast([128, E]),
                                        in1=eidx[:], op=ALU.is_equal)
                mg = wpool.tile([128, E], F32, tag="mg", bufs=G + 1,
                                name=f"mg{g}")
                nc.vector.tensor_add(mg[:], m0g[:], m1g[:])
                mgs.append(mg)
                pwg = psH.tile([128, 512], F32, tag="hpsa", name="pwg",
                               bufs=2)[:, :E]
                nc.tensor.matmul(pwg[:], lhsT=ustrict[:], rhs=mg[:],
                                 start=True, stop=False)
                for gp in range(g):
                    nc.tensor.matmul(pwg[:], lhsT=onesmat[:], rhs=mgs[gp][:],
                                     start=False, stop=False)
                nc.tensor.matmul(pwg[:], lhsT=ones_row1[:], rhs=basecap8[:, :],
                                 start=False, stop=True)
                for k, mk in ((0, m0g), (1, m1g)):
                    pk = wpool.tile([128, E], F32, tag="pk", bufs=5)
                    nc.vector.tensor_mul(pk[:], pwg[:], mk[:])
                    with nc.allow_low_precision(reason="integer-valued positions"):
                        nc.vector.tensor_reduce(pos_i[k][:, g:g + 1], pk[:],
                                                axis=mybir.AxisListType.X,
                                                op=ALU.add)
                    nc.gpsimd.indirect_dma_start(
                        out=meta2[:, :],
                        out_offset=bass.IndirectOffsetOnAxis(
                            ap=pos_i[k][:, g:g + 1], axis=0),
                        in_=pairs[k][:, g, :],
                        in_offset=None,
                    )

            meta_sb = rpool.tile([128, NST], I32)
            nc.sync.dma_start(
                out=meta_sb[:],
                in_=meta2[:, 0:1].rearrange("(s q) one -> q (s one)", q=128))
            wcol = rpool.tile([128, E, ST], F32)
            nc.sync.dma_start(
                out=wcol[:].bitcast(I32),
                in_=meta2[:, 1:2].rearrange("(e s q) one -> q e (s one)",
                                            q=128, s=ST))

            # ---------------- Phase E: software-pipelined expert stages ----
            xTs, hss, xgss, eixs = {}, {}, {}, {}

            def emit_A0(e):
                xT = xtpool.tile([128, 8, CAP], F8, tag="xT", name=f"xT{e}")
                eix = wpool.tile([128, ST], I32, tag="eix", bufs=1 + QUAD,
                                 name=f"eix{e}")
                nc.vector.tensor_scalar_add(
                    eix[:], meta_sb[:, e * ST:(e + 1) * ST], float(e * T))
                eixs[e] = eix
                xgs = []
                for s in range(ST):
                    xg = xgpool.tile([128, 1024], F16, tag="xg", name=f"xg{e}_{s}")
                    nc.gpsimd.indirect_dma_start(
                        out=xg[:], out_offset=None, in_=xb[:, :],
                        in_offset=bass.IndirectOffsetOnAxis(
                            ap=eix[:, s:s + 1], axis=0),
                    )
                    xgs.append(xg)
                    tp = psT.tile([128, 1024], F16, tag="tp", bufs=1, name="tp")
                    for dc in range(8):
                        nc.tensor.transpose(tp[:, dc * 128:(dc + 1) * 128],
                                            xg[:, dc * 128:(dc + 1) * 128],
                                            ident16[:])
                    cp_out = xT[:, :, s * 128:(s + 1) * 128]
                    cp_in = tp[:].rearrange("p (dc t) -> p dc t", dc=8)
                    if s % 2 == 1:
                        nc.vector.tensor_copy(cp_out, cp_in)
                    else:
                        nc.scalar.copy(cp_out, cp_in)
                xTs[e] = xT
                xgss[e] = xgs

            def emit_A1(e):
                xT, w1_sb = xTs[e], w1_sbs[e]
                h_sb = hpool.tile([128, 4, CAP], F8, tag="h_sb", name=f"h{e}")
                hss[e] = h_sb
                for hc in range(4):
                    hpsa = psH.tile([128, 512], F32, tag="hpsa", bufs=2,
                                    name=f"hpsa{e}_{hc}")
                    for c in range(4):
                        nc.tensor.matmul(
                            hpsa[:],
                            lhsT=w1_sb[:, 2 * c:2 * c + 2,
                                       hc * 128:(hc + 1) * 128],
                            rhs=xT[:, 2 * c:2 * c + 2, 0:512],
                            start=(c == 0), stop=(c == 3),
                            perf_mode=PM.DoubleRow)
                    hpsb = psH.tile([128, 128], F32, tag="hpsb", bufs=1,
                                    name=f"hpsb{e}_{hc}")
                    for c in range(4):
                        nc.tensor.matmul(
                            hpsb[:],
                            lhsT=w1_sb[:, 2 * c:2 * c + 2,
                                       hc * 128:(hc + 1) * 128],
                            rhs=xT[:, 2 * c:2 * c + 2, 512:CAP],
                            start=(c == 0), stop=(c == 3),
                            perf_mode=PM.DoubleRow)
                    nc.scalar.activation(h_sb[:, hc, 0:512], hpsa[:],
                                         AF.Gelu, bias=b1_sb[:, e, hc:hc + 1],
                                         scale=1.0)
                    nc.scalar.activation(h_sb[:, hc, 512:CAP], hpsb[:],
                                         AF.Gelu, bias=b1_sb[:, e, hc:hc + 1],
                                         scale=1.0)

            def emit_B(e):
                w2_sb = w2_sbs[e]
                h_sb = hss[e]
                variant2 = (e % 2 == 1)
                if variant2:
                    w2s_sb = wtpool.tile([128, 4, 1], F8, tag="w2s_sb",
                                         bufs=2, name=f"w2s_sb{e}")
                    nc.scalar.dma_start(out=w2s_sb[:], in_=w2s[e])
                sy5 = wpool.tile([128, ST], F32, tag="sy5", bufs=2, name="sy5")
                ss5 = wpool.tile([128, ST], F32, tag="ss5", bufs=2, name="ss5")
                negmu5 = wpool.tile([128, ST], F32, tag="negmu5", bufs=2,
                                    name="negmu5")
                y_sbs = []
                for s in range(ST):
                    yps = psY.tile([128, D], F32, tag="yps", name="yps")
                    for nch in range(2):
                        for c in range(2):
                            nc.tensor.matmul(
                                yps[:, nch * 512:(nch + 1) * 512],
                                lhsT=h_sb[:, 2 * c:2 * c + 2,
                                          s * 128:(s + 1) * 128],
                                rhs=w2_sb[:, 2 * c:2 * c + 2,
                                          nch * 512:(nch + 1) * 512],
                                start=(c == 0), stop=(c == 1),
                                perf_mode=PM.DoubleRow)
                    y_sb = wpool.tile([128, D], F16, tag="y_sb", bufs=ST + 1,
                                      name="y_sb")
                    if variant2:
                        ymu = psH.tile([128, 128], F32, tag="hpsb", bufs=1,
                                       name="ymu")[:, 0:1]
                        for c in range(2):
                            nc.tensor.matmul(
                                ymu[:],
                                lhsT=h_sb[:, 2 * c:2 * c + 2,
                                          s * 128:(s + 1) * 128],
                                rhs=w2s_sb[:, 2 * c:2 * c + 2, :],
                                start=(c == 0), stop=False,
                                perf_mode=PM.DoubleRow)
                        for c in range(4):
                            nc.tensor.matmul(
                                ymu[:],
                                lhsT=xTs[e][:, 2 * c:2 * c + 2,
                                            s * 128:(s + 1) * 128],
                                rhs=ones8c[:],
                                start=False, stop=(c == 3),
                                perf_mode=PM.DoubleRow)
                        nc.scalar.copy(y_sb[:], yps[:])
                        nc.gpsimd.indirect_dma_start(
                            out=y_sb[:], out_offset=None, in_=xb[:, :],
                            in_offset=bass.IndirectOffsetOnAxis(
                                ap=eixs[e][:, s:s + 1], axis=0),
                            compute_op=ALU.add,
                        )
                        nc.vector.tensor_scalar_mul(
                            negmu5[:, s:s + 1], ymu[:], -1.0 / D)
                    else:
                        nc.vector.scalar_tensor_tensor(
                            out=y_sb[:], in0=yps[:], scalar=0.0,
                            in1=xgss[e][s][:, 0:1024],
                            op0=ALU.add, op1=ALU.add,
                            accum_out=sy5[:, s:s + 1])
                    sqscr = wpool.tile([128, D], F16, tag="sqscr", bufs=3,
                                       name="sqscr")
                    if (e + s) % 2 == 0:
                        nc.scalar.activation(sqscr[:], y_sb[:], AF.Square,
                                             accum_out=ss5[:, s:s + 1])

                    else:
                        nc.vector.scalar_tensor_tensor(
                            out=sqscr[:], in0=y_sb[:], scalar=0.0,
                            in1=y_sb[:], op0=ALU.add, op1=ALU.mult,
                            accum_out=ss5[:, s:s + 1])
                    y_sbs.append(y_sb)
                if not variant2:
                    nc.vector.tensor_scalar_mul(negmu5[:], sy5[:], -1.0 / D)
                msq = wpool.tile([128, ST], F32, tag="msq", name="msq")
                nc.vector.tensor_mul(msq[:], negmu5[:], negmu5[:])
                varp = wpool.tile([128, ST], F32, tag="varp", name="varp")
                nc.vector.scalar_tensor_tensor(
                    out=varp[:], in0=ss5[:], scalar=1.0 / D, in1=msq[:],
                    op0=ALU.mult, op1=ALU.subtract)
                vh5 = wpool.tile([128, ST], F32, tag="vh5", name="vh5")
                nc.vector.tensor_scalar(vh5[:], varp[:], 0.5, 0.5 * LN_EPS,
                                        op0=ALU.mult, op1=ALU.add)
                ib = wpool.tile([128, ST], I32, tag="ib", name="ib")
                nc.vector.tensor_scalar(ib[:], vh5[:].bitcast(I32), 1, 0,
                                        op0=ALU.logical_shift_right,
                                        op1=ALU.logical_shift_right)
                yb = wpool.tile([128, ST], I32, tag="yb", name="yb")
                nc.vector.tensor_tensor(out=yb[:],
                                        in0=magic_col[:].to_broadcast([128, ST]),
                                        in1=ib[:], op=ALU.subtract)
                rstd5 = yb[:].bitcast(F32)
                for _ in range(1):
                    ya = wpool.tile([128, ST], F32, tag="ya", name="ya")
                    nc.vector.tensor_mul(ya[:], rstd5, rstd5)
                    nc.vector.tensor_mul(ya[:], ya[:], vh5[:])
                    nc.vector.tensor_scalar(ya[:], ya[:], -1.0, 1.5,
                                            op0=ALU.mult, op1=ALU.add)
                    nc.vector.tensor_mul(rstd5, rstd5, ya[:])
                rsw5 = wpool.tile([128, ST], F32, tag="rsw5", bufs=2, name="rsw5")
                nc.vector.tensor_mul(rsw5[:], rstd5, wcol[:, e, :])
                for s in range(ST):
                    S = e * ST + s
                    rn = wpool.tile([128, D], F16, tag="rn", bufs=4, name="rn")
                    nc.vector.tensor_scalar(
                        rn[:], y_sbs[s][:], negmu5[:, s:s + 1],
                        rsw5[:, s:s + 1], op0=ALU.add, op1=ALU.mult)
                    nc.gpsimd.indirect_dma_start(
                        out=outb[:, :],
                        out_offset=bass.IndirectOffsetOnAxis(
                            ap=meta_sb[:, S:S + 1], axis=0),
                        in_=rn[:], in_offset=None,
                        compute_op=ALU.add,
                    )

            def prefetch_weights(ne):
                w1_sbs[ne] = wtpool.tile([128, 8, H], F8,
                                         tag=f"w1_sb{ne % QUAD}",
                                         name=f"w1s{ne}")
                nc.sync.dma_start(out=w1_sbs[ne][:], in_=w1[ne])
                w2_sbs[ne] = wtpool.tile([128, 4, D], F8,
                                         tag=f"w2_sb{ne % QUAD}",
                                         name=f"w2s{ne}")
                nc.sync.dma_start(out=w2_sbs[ne][:], in_=w2[ne])

            emit_A0(0)
            emit_A0(1)
            emit_A1(0)
            for e in range(E):
                if e + 2 < E:
                    if e + 2 >= QUAD:
                        prefetch_weights(e + 2)
                    emit_A0(e + 2)
                if e + 1 < E:
                    emit_A1(e + 1)
                emit_B(e)

    _legalize_multiwait(nc)
    return nc


def make_in_maps(inputs):
    x = np.ascontiguousarray(
        np.asarray(inputs["x"], dtype=np.float32).reshape(-1, D))
    Wg = np.asarray(inputs["Wg"], dtype=np.float32)
    bgv = np.asarray(inputs["bg"], dtype=np.float32)
    W1 = np.asarray(inputs["W1"], dtype=np.float32)
    b1 = np.asarray(inputs["b1"], dtype=np.float32)
    W2 = np.asarray(inputs["W2"], dtype=np.float32)
    b2v = np.asarray(inputs["b2"], dtype=np.float32)

    wgt = np.ascontiguousarray(Wg.reshape(8, 128, E).transpose(1, 0, 2))
    # b1' = b1 - b2 @ W1  (compensates the b2 shift folded into the gather)
    b1p = b1 - np.einsum('ed,edh->eh', b2v, W1)
    b1t = np.ascontiguousarray(b1p.reshape(E, 4, 128).transpose(2, 0, 1))
    w1f8 = np.ascontiguousarray(
        W1.reshape(E, 8, 128, H).transpose(0, 2, 1, 3)).astype(md.float8_e4m3)
    w2f8 = np.ascontiguousarray(
        W2.reshape(E, 4, 128, D).transpose(0, 2, 1, 3)).astype(md.float8_e4m3)
    w2sum = np.ascontiguousarray(
        W2.sum(axis=2).reshape(E, 4, 128, 1).transpose(0, 2, 1, 3)
    ).astype(md.float8_e4m3)

    shared = {
        "wgt": wgt,
        "bg": bgv.reshape(1, E),
        "w1": w1f8,
        "b1t": b1t,
        "w2": w2f8,
        "w2s": w2sum,
        "zrow": np.zeros((1, D), np.float16),
    }
    maps = []
    for c in range(N_CORES):
        xs = x[c * T:(c + 1) * T]
        xtc = np.ascontiguousarray(xs.reshape(T, 8, 128).transpose(2, 1, 0))
        xbe = np.zeros((E * T, XROW), np.float16)
        for e in range(E):
            xbe[e * T:(e + 1) * T] = (xs + b2v[e]).astype(np.float16)
        maps.append(dict(shared, xt=xtc, xb=xbe))
    return maps


_CACHED = {}


def kernel(**inputs):
    _apply_tile_patch()
    from concourse.bass_utils import run_bass_kernel_spmd

    if "nc" not in _CACHED:
        _CACHED["nc"] = build_kernel()
    nc = _CACHED["nc"]
    in_maps = make_in_maps(inputs)
    res = run_bass_kernel_spmd(nc, in_maps, core_ids=list(range(N_CORES)),
                               trace=False)
    out = np.concatenate(
        [np.asarray(res.results[c]["outb"]).astype(np.float32)
         for c in range(N_CORES)], axis=0)
    xshape = np.asarray(inputs["x"]).shape
    return out.reshape(xshape)
